# revision 14
# baseline (speedup 1.0000x reference)
"""HONAM (order-2 Newton-identity NAM) Trainium2 kernel.

Math: with zero biases (guaranteed by the input spec: b1/b2/b3 fill zeros),
each per-feature MLP t[b,f,:] = relu-net(x[b,f]) is positively homogeneous
on each half-line of its scalar input:
    x > 0:  t = x * P3[f],   P3 = relu(relu(relu(W1)@W2)@W3)
    x < 0:  t = (-x) * N3[f],  N3 = same chain from -W1
Hence with U = relu(x), V = relu(-x) (U*V == 0 elementwise):
    p1 = U @ P3 + V @ N3
    p2 = sum_f t^2 = U^2 @ (P3*P3) + V^2 @ (N3*N3)
        where U^2 = x*U and V^2 = -x*V  (signs folded into the weights)
    pred = p1 @ Wout[:32] + (p1^2 - p2)/2 @ Wout[32:] + bout

Device kernel per core (batch-sharded 8192/8 = 1024 rows):
    PE-transpose x -> 4 activation streams -> 8 thin K=128 fp32 matmuls
    (p1/p2 PSUM accumulation) -> tiny tail -> [1024,1] output.
"""

import numpy as np

B, F, H1, H2, H3 = 8192, 256, 32, 64, 32
NCORES = 8
BC = B // NCORES          # 1024 batch rows per core
CHUNK = 512               # batch columns per PSUM bank / matmul
NSUB = CHUNK // 128       # 128-row subtiles per chunk

_compiled = None


def _build():
    import concourse.bacc as bacc
    import concourse.mybir as mybir
    import concourse.bass as bass
    import concourse.tile as tile
    from concourse.masks import make_identity
    from contextlib import ExitStack

    fp32 = mybir.dt.float32
    AF = mybir.ActivationFunctionType

    nc = bacc.Bacc("TRN2", target_bir_lowering=False, debug=False,
                   num_devices=NCORES)

    xc = nc.dram_tensor("xc", [BC, F], fp32, kind="ExternalInput").ap()
    lhs = nc.dram_tensor("lhs", [128, 256], mybir.dt.float32r, kind="ExternalInput").ap()
    wout2 = nc.dram_tensor("wout2", [2 * H3, 1], fp32, kind="ExternalInput").ap()
    boutt = nc.dram_tensor("boutt", [1, 1], fp32, kind="ExternalInput").ap()
    y = nc.dram_tensor("y", [BC, 1], fp32, kind="ExternalOutput").ap()

    f32r = mybir.dt.float32r

    with tile.TileContext(nc) as tc:
        with ExitStack() as ctx:
            consts = ctx.enter_context(tc.tile_pool(name="consts", bufs=1))
            xin = ctx.enter_context(tc.tile_pool(name="xin", bufs=2))
            sb = ctx.enter_context(tc.tile_pool(name="sb", bufs=2))
            tail = ctx.enter_context(tc.tile_pool(name="tail", bufs=2))
            psum = ctx.enter_context(
                tc.tile_pool(name="psum", bufs=2, space=bass.MemorySpace.PSUM))
            acc = ctx.enter_context(
                tc.tile_pool(name="acc", bufs=2, space=bass.MemorySpace.PSUM))

            ident = consts.tile([128, 128], fp32, tag="ident")
            make_identity(nc, ident[:])
            lhs_t = consts.tile([128, 256], f32r, tag="lhs")
            nc.sync.dma_start(lhs_t[:], lhs[:])
            wout_t = consts.tile([2 * H3, 1], fp32, tag="wout")
            nc.sync.dma_start(wout_t[:], wout2[:])
            bout_t = consts.tile([1, 1], fp32, tag="bout")
            nc.sync.dma_start(bout_t[:], boutt[:])

            y2 = y.rearrange("(a b) m -> a (b m)", b=CHUNK)  # [2, 512] view
            # lhs pack column layout: 8 blocks of 32:
            # P3a P3b N3a N3b Qpa Qpb Qna Qnb
            def LHS(i):
                return lhs_t[:, 32 * i:32 * (i + 1)]

            # prefetch both chunks' x upfront, one DMA each, two queues
            xbig = []
            for c in range(BC // CHUNK):
                xb = xin.tile([128, NSUB * F], fp32, tag=f"x{c}", name=f"xb{c}")
                src = xc[c * CHUNK:(c + 1) * CHUNK, :].rearrange(
                    "(s p) f -> p s f", p=128)
                dst = xb[:].rearrange("p (s f) -> p s f", s=NSUB)
                (nc.sync if c == 0 else nc.scalar).dma_start(dst, src)
                xbig.append(xb)

            for c in range(BC // CHUNK):
                # ---- transpose: xT [128, 2*CHUNK] in PSUM ----
                # cols [fh*CHUNK + s*128 : +128] = x[c*CHUNK+s*128.., fh*128..]^T
                xt = psum.tile([128, 2 * CHUNK], fp32, tag="xt", name=f"xt{c}")
                for s in range(NSUB):
                    for fh in range(2):
                        nc.tensor.transpose(
                            xt[:, fh * CHUNK + 128 * s: fh * CHUNK + 128 * (s + 1)],
                            xbig[c][:, 256 * s + 128 * fh: 256 * s + 128 * (fh + 1)],
                            ident[:])

                # ---- activation streams (FD=1024 ops over both K-halves) ----
                u = sb.tile([128, 2 * CHUNK], f32r, tag="u", name=f"u{c}")
                v = sb.tile([128, 2 * CHUNK], f32r, tag="v", name=f"v{c}")
                su = sb.tile([128, 2 * CHUNK], f32r, tag="su", name=f"su{c}")
                sv = sb.tile([128, 2 * CHUNK], f32r, tag="sv", name=f"sv{c}")
                nc.scalar.activation(u[:], xt[:], AF.Relu)
                nc.scalar.activation(v[:], xt[:], AF.Relu, scale=-1.0)
                nc.vector.tensor_mul(su[:], xt[:], u[:])   # x*relu(x)  = U^2
                nc.vector.tensor_mul(sv[:], xt[:], v[:])   # x*relu(-x) = -V^2

                # ---- p1/p2 matmuls (K=128 each, f32r single-pass, PSUM acc) ----
                def sl(t, h):
                    return t[:, h * CHUNK:(h + 1) * CHUNK]

                p1 = acc.tile([H3, CHUNK], fp32, tag="p1")
                p2 = acc.tile([H3, CHUNK], fp32, tag="p2")
                nc.tensor.matmul(p1[:], LHS(0), sl(u, 0), start=True, stop=False)
                nc.tensor.matmul(p1[:], LHS(1), sl(u, 1), start=False, stop=False)
                nc.tensor.matmul(p1[:], LHS(2), sl(v, 0), start=False, stop=False)
                nc.tensor.matmul(p1[:], LHS(3), sl(v, 1), start=False, stop=True)
                nc.tensor.matmul(p2[:], LHS(4), sl(su, 0), start=True, stop=False)
                nc.tensor.matmul(p2[:], LHS(5), sl(su, 1), start=False, stop=False)
                nc.tensor.matmul(p2[:], LHS(6), sl(sv, 0), start=False, stop=False)
                nc.tensor.matmul(p2[:], LHS(7), sl(sv, 1), start=False, stop=True)

                # ---- tail: feats = [p1 ; p1^2 - p2], pred = wout2.T @ feats ----
                feats = tail.tile([2 * H3, CHUNK], fp32, tag="feats")
                nc.scalar.activation(feats[0:H3, :], p1[:], AF.Copy)
                sq = tail.tile([H3, CHUNK], fp32, tag="sq")
                nc.vector.tensor_mul(sq[:], feats[0:H3, :], feats[0:H3, :])
                nc.vector.tensor_sub(feats[H3:2 * H3, :], sq[:], p2[:])
                pred_ps = psum.tile([128, 2 * CHUNK], fp32, tag="xt",
                                    name=f"pred_ps{c}")
                nc.tensor.matmul(pred_ps[0:1, 0:CHUNK], wout_t[:],
                                 feats[:], start=True, stop=True)
                pred_sb = tail.tile([1, CHUNK], fp32, tag="pred")
                nc.scalar.activation(pred_sb[:], pred_ps[0:1, 0:CHUNK], AF.Identity,
                                     bias=bout_t[:])
                nc.sync.dma_start(y2[c:c + 1, :], pred_sb[:])

    nc.compile()
    return nc


def _get_compiled():
    global _compiled
    if _compiled is None:
        _compiled = _build()
    return _compiled


def _reference_numpy(x, W1, b1, W2, b2, W3, b3, Wout, bout):
    # Safety net for nonzero biases (never hit with the spec'd inputs).
    h = np.maximum(x[:, :, None] * W1[:, 0, :][None] + b1[None], 0.0)
    h = np.maximum(np.einsum('bfh,fho->bfo', h, W2) + b2[None], 0.0)
    t = np.maximum(np.einsum('bfh,fho->bfo', h, W3) + b3[None], 0.0)
    p1 = t.sum(axis=1)
    p2 = (t * t).sum(axis=1)
    feats = np.concatenate([p1, (p1 * p1 - p2) * 0.5], axis=1)
    return (feats @ Wout + bout).astype(np.float32)


def kernel(x, W1, b1, W2, b2, W3, b3, Wout, bout):
    x = np.asarray(x, dtype=np.float32)
    W1 = np.asarray(W1, dtype=np.float32)
    W2 = np.asarray(W2, dtype=np.float32)
    W3 = np.asarray(W3, dtype=np.float32)
    Wout = np.asarray(Wout, dtype=np.float32)
    bout = np.asarray(bout, dtype=np.float32)
    if np.any(b1) or np.any(b2) or np.any(b3):
        return _reference_numpy(x, W1, b1, W2, b2, W3, b3, Wout, bout)

    P1 = np.maximum(W1[:, 0, :], 0.0)
    N1 = np.maximum(-W1[:, 0, :], 0.0)
    P3 = np.maximum(np.einsum('fh,fho->fo',
                    np.maximum(np.einsum('fh,fho->fo', P1, W2), 0.0), W3), 0.0)
    N3 = np.maximum(np.einsum('fh,fho->fo',
                    np.maximum(np.einsum('fh,fho->fo', N1, W2), 0.0), W3), 0.0)
    Qp = P3 * P3
    Qn = -(N3 * N3)
    lhs = np.concatenate(
        [P3[:128], P3[128:], N3[:128], N3[128:],
         Qp[:128], Qp[128:], Qn[:128], Qn[128:]], axis=1).astype(np.float32)
    wout2 = np.concatenate([Wout[:H3], Wout[H3:] * 0.5], axis=0).astype(np.float32)
    boutt = bout.reshape(1, 1).astype(np.float32)

    import concourse.bass_utils as bass_utils
    nc = _get_compiled()
    in_maps = [
        {"xc": np.ascontiguousarray(x[i * BC:(i + 1) * BC]),
         "lhs": lhs, "wout2": wout2, "boutt": boutt}
        for i in range(NCORES)
    ]
    res = bass_utils.run_bass_kernel_spmd(nc, in_maps, core_ids=list(range(NCORES)))
    out = np.concatenate([res.results[i]["y"] for i in range(NCORES)], axis=0)
    return out.astype(np.float32)


# revision 16
# speedup vs baseline: 1.4017x; 1.4017x over previous
"""HONAM (order-2 Newton-identity NAM) Trainium2 kernel.

Math: with zero biases (guaranteed by the input spec: b1/b2/b3 fill zeros),
each per-feature MLP t[b,f,:] = relu-net(x[b,f]) is positively homogeneous
on each half-line of its scalar input:
    x > 0:  t = x * P3[f],   P3 = relu(relu(relu(W1)@W2)@W3)
    x < 0:  t = (-x) * N3[f],  N3 = same chain from -W1
Hence with U = relu(x), V = relu(-x) (U*V == 0 elementwise):
    p1 = U @ P3 + V @ N3
    p2 = sum_f t^2 = U^2 @ (P3*P3) + V^2 @ (N3*N3)
        where U^2 = x*U and V^2 = -x*V  (signs folded into the weights)
    pred = p1 @ Wout[:32] + (p1^2 - p2)/2 @ Wout[32:] + bout

Device kernel per core (batch-sharded 8192/8 = 1024 rows):
    PE-transpose x -> 4 activation streams -> 8 thin K=128 fp32 matmuls
    (p1/p2 PSUM accumulation) -> tiny tail -> [1024,1] output.
"""

import numpy as np

B, F, H1, H2, H3 = 8192, 256, 32, 64, 32
NCORES = 8
BC = B // NCORES          # 1024 batch rows per core
CHUNK = 512               # batch columns per PSUM bank / matmul
NSUB = CHUNK // 128       # 128-row subtiles per chunk

_compiled = None


def _build():
    import concourse.bacc as bacc
    import concourse.mybir as mybir
    import concourse.bass as bass
    import concourse.tile as tile
    from concourse.masks import make_identity
    from contextlib import ExitStack

    fp32 = mybir.dt.float32
    AF = mybir.ActivationFunctionType

    nc = bacc.Bacc("TRN2", target_bir_lowering=False, debug=False,
                   num_devices=NCORES)

    xc = nc.dram_tensor("xc", [BC, F], fp32, kind="ExternalInput").ap()
    lhs = nc.dram_tensor("lhs", [128, 256], mybir.dt.float32r, kind="ExternalInput").ap()
    wout2 = nc.dram_tensor("wout2", [2 * H3, 1], mybir.dt.float32r, kind="ExternalInput").ap()
    boutt = nc.dram_tensor("boutt", [1, 1], fp32, kind="ExternalInput").ap()
    y = nc.dram_tensor("y", [BC, 1], fp32, kind="ExternalOutput").ap()

    f32r = mybir.dt.float32r

    with tile.TileContext(nc) as tc:
        with ExitStack() as ctx:
            consts = ctx.enter_context(tc.tile_pool(name="consts", bufs=1))
            xin = ctx.enter_context(tc.tile_pool(name="xin", bufs=2))
            sb = ctx.enter_context(tc.tile_pool(name="sb", bufs=2))
            tail = ctx.enter_context(tc.tile_pool(name="tail", bufs=2))
            psum = ctx.enter_context(
                tc.tile_pool(name="psum", bufs=2, space=bass.MemorySpace.PSUM))
            acc = ctx.enter_context(
                tc.tile_pool(name="acc", bufs=2, space=bass.MemorySpace.PSUM))

            # x loads first: they gate the PE pipeline
            xbig = []
            for c in range(BC // CHUNK):
                xb = xin.tile([128, NSUB * F], fp32, tag=f"x{c}", name=f"xb{c}")
                src = xc[c * CHUNK:(c + 1) * CHUNK, :].rearrange(
                    "(s p) f -> p s f", p=128)
                dst = xb[:].rearrange("p (s f) -> p s f", s=NSUB)
                (nc.sync if c == 0 else nc.scalar).dma_start(dst, src)
                xbig.append(xb)

            ident = consts.tile([128, 128], fp32, tag="ident")
            make_identity(nc, ident[:])
            lhs_t = consts.tile([128, 256], f32r, tag="lhs")
            nc.scalar.dma_start(lhs_t[:], lhs[:])
            wout_t = consts.tile([2 * H3, 1], f32r, tag="wout")
            nc.scalar.dma_start(wout_t[:], wout2[:])
            bout_t = consts.tile([1, 1], fp32, tag="bout")
            nc.scalar.dma_start(bout_t[:], boutt[:])

            y2 = y.rearrange("(a b) m -> a (b m)", b=CHUNK)  # [2, 512] view
            # lhs pack column layout: 8 blocks of 32:
            # P3a P3b N3a N3b Qpa Qpb Qna Qnb
            def LHS(i):
                return lhs_t[:, 32 * i:32 * (i + 1)]

            P1L, P2L = [], []
            for c in range(BC // CHUNK):
                # ---- transpose: xT [128, 2*CHUNK] in PSUM ----
                # cols [fh*CHUNK + s*128 : +128] = x[c*CHUNK+s*128.., fh*128..]^T
                xt = psum.tile([128, 2 * CHUNK], fp32, tag="xt", name=f"xt{c}")
                for s in range(NSUB):
                    for fh in range(2):
                        nc.tensor.transpose(
                            xt[:, fh * CHUNK + 128 * s: fh * CHUNK + 128 * (s + 1)],
                            xbig[c][:, 256 * s + 128 * fh: 256 * s + 128 * (fh + 1)],
                            ident[:])

                # ---- activation streams (FD=1024 ops over both K-halves) ----
                u = sb.tile([128, 2 * CHUNK], f32r, tag="u", name=f"u{c}")
                v = sb.tile([128, 2 * CHUNK], f32r, tag="v", name=f"v{c}")
                su = sb.tile([128, 2 * CHUNK], f32r, tag="su", name=f"su{c}")
                sv = sb.tile([128, 2 * CHUNK], f32r, tag="sv", name=f"sv{c}")
                nc.scalar.activation(u[:], xt[:], AF.Relu)
                nc.scalar.activation(v[:], xt[:], AF.Relu, scale=-1.0)
                nc.vector.tensor_mul(su[:], xt[:], u[:])   # x*relu(x)  = U^2
                nc.vector.tensor_mul(sv[:], xt[:], v[:])   # x*relu(-x) = -V^2

                # ---- p1/p2 matmuls (K=128 each, f32r single-pass, PSUM acc) ----
                def sl(t, h):
                    return t[:, h * CHUNK:(h + 1) * CHUNK]

                p1 = acc.tile([H3, CHUNK], fp32, tag="p1")
                p2 = acc.tile([H3, CHUNK], fp32, tag="p2")
                nc.tensor.matmul(p1[:], LHS(0), sl(u, 0), start=True, stop=False)
                nc.tensor.matmul(p1[:], LHS(1), sl(u, 1), start=False, stop=False)
                nc.tensor.matmul(p1[:], LHS(2), sl(v, 0), start=False, stop=False)
                nc.tensor.matmul(p1[:], LHS(3), sl(v, 1), start=False, stop=True)
                nc.tensor.matmul(p2[:], LHS(4), sl(su, 0), start=True, stop=False)
                nc.tensor.matmul(p2[:], LHS(5), sl(su, 1), start=False, stop=False)
                nc.tensor.matmul(p2[:], LHS(6), sl(sv, 0), start=False, stop=False)
                nc.tensor.matmul(p2[:], LHS(7), sl(sv, 1), start=False, stop=True)
                P1L.append(p1)
                P2L.append(p2)

            for c in range(BC // CHUNK):
                p1, p2 = P1L[c], P2L[c]
                # ---- tail: feats = [p1 ; p1^2 - p2], pred = wout2.T @ feats ----
                feats = tail.tile([2 * H3, CHUNK], f32r, tag="feats",
                  name=f"feats{c}")
                nc.scalar.activation(feats[0:H3, :], p1[:], AF.Copy)
                sq = tail.tile([H3, CHUNK], fp32, tag="sq", name=f"sq{c}")
                nc.vector.tensor_mul(sq[:], feats[0:H3, :], feats[0:H3, :])
                nc.vector.tensor_sub(feats[H3:2 * H3, :], sq[:], p2[:])
                pred_ps = psum.tile([128, 2 * CHUNK], fp32, tag="xt",
                                    name=f"pred_ps{c}")
                nc.tensor.matmul(pred_ps[0:1, 0:CHUNK], wout_t[:],
                                 feats[:], start=True, stop=True)
                pred_sb = tail.tile([1, CHUNK], fp32, tag="pred")
                nc.scalar.activation(pred_sb[:], pred_ps[0:1, 0:CHUNK], AF.Identity,
                                     bias=bout_t[:])
                nc.sync.dma_start(y2[c:c + 1, :], pred_sb[:])

    nc.compile()
    return nc


def _get_compiled():
    global _compiled
    if _compiled is None:
        _compiled = _build()
    return _compiled


def _reference_numpy(x, W1, b1, W2, b2, W3, b3, Wout, bout):
    # Safety net for nonzero biases (never hit with the spec'd inputs).
    h = np.maximum(x[:, :, None] * W1[:, 0, :][None] + b1[None], 0.0)
    h = np.maximum(np.einsum('bfh,fho->bfo', h, W2) + b2[None], 0.0)
    t = np.maximum(np.einsum('bfh,fho->bfo', h, W3) + b3[None], 0.0)
    p1 = t.sum(axis=1)
    p2 = (t * t).sum(axis=1)
    feats = np.concatenate([p1, (p1 * p1 - p2) * 0.5], axis=1)
    return (feats @ Wout + bout).astype(np.float32)


def kernel(x, W1, b1, W2, b2, W3, b3, Wout, bout):
    x = np.asarray(x, dtype=np.float32)
    W1 = np.asarray(W1, dtype=np.float32)
    W2 = np.asarray(W2, dtype=np.float32)
    W3 = np.asarray(W3, dtype=np.float32)
    Wout = np.asarray(Wout, dtype=np.float32)
    bout = np.asarray(bout, dtype=np.float32)
    if np.any(b1) or np.any(b2) or np.any(b3):
        return _reference_numpy(x, W1, b1, W2, b2, W3, b3, Wout, bout)

    P1 = np.maximum(W1[:, 0, :], 0.0)
    N1 = np.maximum(-W1[:, 0, :], 0.0)
    P3 = np.maximum(np.einsum('fh,fho->fo',
                    np.maximum(np.einsum('fh,fho->fo', P1, W2), 0.0), W3), 0.0)
    N3 = np.maximum(np.einsum('fh,fho->fo',
                    np.maximum(np.einsum('fh,fho->fo', N1, W2), 0.0), W3), 0.0)
    Qp = P3 * P3
    Qn = -(N3 * N3)
    lhs = np.concatenate(
        [P3[:128], P3[128:], N3[:128], N3[128:],
         Qp[:128], Qp[128:], Qn[:128], Qn[128:]], axis=1).astype(np.float32)
    wout2 = np.concatenate([Wout[:H3], Wout[H3:] * 0.5], axis=0).astype(np.float32)
    boutt = bout.reshape(1, 1).astype(np.float32)

    import concourse.bass_utils as bass_utils
    nc = _get_compiled()
    in_maps = [
        {"xc": np.ascontiguousarray(x[i * BC:(i + 1) * BC]),
         "lhs": lhs, "wout2": wout2, "boutt": boutt}
        for i in range(NCORES)
    ]
    res = bass_utils.run_bass_kernel_spmd(nc, in_maps, core_ids=list(range(NCORES)))
    out = np.concatenate([res.results[i]["y"] for i in range(NCORES)], axis=0)
    return out.astype(np.float32)
